# revision 18
# baseline (speedup 1.0000x reference)
"""Trainium2 Bass kernel for nn_DecoderModel (12-layer decoder w/ image token).

Sharding: TP-4 x DP-2.
  - cores 0-3 handle batch 0, cores 4-7 batch 1 (data parallel over batch)
  - within each group of 4: Megatron TP (4 heads/core, qkv col-sharded,
    proj row-sharded, fc col-sharded 1024 dff/core, fc2 row-sharded)
  - AllReduce on replica groups [[0,1,2,3],[4,5,6,7]] -- the two groups'
    collectives run concurrently on disjoint links
  - lm head: vocab quarter per core within each batch group
  - tokens of each batch split in two 256-halves, software-pipelined so
    each AllReduce hides behind the other half's compute

Layout: residual kept feature-major (h^T: [D, tok]); LayerNorm folded into
matmuls: y = r .* (x @ Wg - mu * colsum(Wg)), Wg = gamma-folded W; the
-mu*colsum term is a K=1 rank-1 matmul into the same PSUM; r rides the
PSUM drain multiply against a PE-broadcast r row.

Attention: kv laid out tokens-first (kt 0..511 = tokens, 512 = image,
pad to 640); scores kt-major so softmax is exp + multiplicative mask;
first token-half only touches kv chunks {0,1,image}. Denominators via a
ones-column appended to transposed V (K=65 AV matmul).

Residual adds ride gpsimd accumulate-DMA (CCE add in the DMA datapath).
"""

import os
import numpy as np

from concourse import bacc, tile, mybir
from concourse import bass_utils

dt = mybir.dt
AF = mybir.ActivationFunctionType
ALU = mybir.AluOpType

# model dims
B, S, D, H, L, V = 2, 512, 1024, 16, 12, 50257
HD = D // H          # 64
DFF = 4 * D          # 4096
NC = 8
TPG = 4              # tensor-parallel group size
HL = H // TPG        # 4 local heads
CW = HL * HD         # 256 local q/k/v cols (2 partition tiles)
DFS = DFF // TPG     # 1024 local dff
SH = S // 2          # 256 tokens per pipeline half
VSH = (V + TPG - 1) // TPG   # 12565 vocab rows per core
NVT = 25                     # vocab tiles of 512
VS = NVT * 512               # 12800 padded vocab shard
EPS = 1e-5

F32 = dt.float32
F16 = dt.float16

RG = [[0, 1, 2, 3], [4, 5, 6, 7]]


def _build(nl, has_bias_qkv, has_bias_proj, has_bias_fc, has_bias_fc2,
           has_bias_lm):
    nc = bacc.Bacc("TRN2", target_bir_lowering=False, debug=False,
                   num_devices=NC)

    dram = lambda n, sh, ty=F32, kind="ExternalInput": nc.dram_tensor(
        n, sh, ty, kind=kind).ap()

    h0T_d = dram("h0T", [D, S], F16)
    wattn_d = dram("wattn", [nl, D, 3 * CW], F16)
    csqkv_d = dram("csqkv", [nl, 1, 3 * CW], F16)
    bqkv_d = dram("bqkv", [nl, 1, 3 * CW], F16) if has_bias_qkv else None
    wproj_d = dram("wproj", [nl, CW, D], F16)
    bproj_d = dram("bproj", [nl, 1, D], F16) if has_bias_proj else None
    wfc_d = dram("wfc", [nl, D, DFS], F16)
    csfc_d = dram("csfc", [nl, 1, DFS], F16)
    bfc_d = dram("bfc", [nl, 1, DFS], F16) if has_bias_fc else None
    wfc2_d = dram("wfc2", [nl, DFS, D], F16)
    bfc2_d = dram("bfc2", [nl, 1, D], F16) if has_bias_fc2 else None
    kivik_d = dram("kivik", [nl, CW, 1], F16)
    kiviv_d = dram("kiviv", [nl, CW, 1], F16)
    mask_d = dram("mask", [5, 128, S], F16)
    ident_d = dram("ident", [128, 128], F16)
    wteT_d = dram("wteT", [D, VS], F16)
    blm_d = dram("blm", [1, VS], F16) if has_bias_lm else None
    logits_d = dram("logits", [S, VS], F16, kind="ExternalOutput")

    with tile.TileContext(nc) as tc:
        with (
            nc.allow_low_precision(reason="f16 matmul pipeline"),
            tc.tile_pool(name="const", bufs=1) as cpool,
            tc.tile_pool(name="resid", bufs=1) as hpool,
            tc.tile_pool(name="rows", bufs=2) as rpool,
            tc.tile_pool(name="dram", bufs=1, space="DRAM") as dpool,
        ):
            ident_sb = cpool.tile([128, 128], F16, name="ident_sb")
            nc.sync.dma_start(ident_sb[:], ident_d[:])
            ones_col = cpool.tile([128, 1], F16, name="ones_col")
            nc.vector.memset(ones_col[:], 1.0)
            ones_row = cpool.tile([1, SH], F16, name="ones_row")
            nc.vector.memset(ones_row[:], 1.0)
            c_eps = cpool.tile([1, 1], F32, name="c_eps")
            nc.vector.memset(c_eps[:], EPS)
            c_ninvD = cpool.tile([1, 1], F32, name="c_ninvD")
            nc.vector.memset(c_ninvD[:], -1.0 / D)

            mask_sb = []
            for c in range(5):
                m = cpool.tile([128, S], F16, name=f"mask_{c}")
                nc.sync.dma_start(m[:], mask_d[c])
                mask_sb.append(m)

            # residual: [kc][hf] tiles [128, 256], feature-major
            hT = []
            for kc in range(8):
                pair = []
                for hf in range(2):
                    t_ = hpool.tile([128, SH], F16, name=f"hT{kc}_{hf}")
                    nc.sync.dma_start(
                        t_[:], h0T_d[kc * 128:(kc + 1) * 128,
                                     hf * SH:(hf + 1) * SH])
                    pair.append(t_)
                hT.append(pair)

            def ln_stats(hf, pfx, sq_pool, ps_row, ps_bc, bc_pool):
                """Stats for one half: returns (rb [128,SH] f32 SBUF bcast,
                nm16 [1,SH] f16, r_row [1,SH] f32)."""
                stat_ps = ps_row.tile([33, SH], F32, tag="stat", bufs=1)
                mu_ps = stat_ps[0:1, :]
                ssq_ps = stat_ps[32:33, :]
                for kc in range(8):
                    nc.tensor.matmul(mu_ps, ones_col[:], hT[kc][hf][:],
                                     start=(kc == 0), stop=(kc == 7))
                for kc in range(8):
                    xsq = sq_pool.tile([128, SH], F16, tag=f"xsq{kc & 1}")
                    nc.gpsimd.tensor_tensor(xsq[:], hT[kc][hf][:],
                                            hT[kc][hf][:], ALU.mult)
                    nc.tensor.matmul(ssq_ps, ones_col[:], xsq[:],
                                     start=(kc == 0), stop=(kc == 7))
                nm16 = rpool.tile([1, SH], F16, tag="nm16",
                                  name=f"nm_{pfx}", bufs=2)
                nc.scalar.mul(nm16[:], mu_ps, c_ninvD[:])
                musq = rpool.tile([1, SH], F32, tag="musq", bufs=2)
                nc.vector.tensor_tensor(musq[:], nm16[:], nm16[:], ALU.mult)
                varr = rpool.tile([1, SH], F32, tag="varr", bufs=2)
                nc.vector.scalar_tensor_tensor(
                    varr[:], ssq_ps, 1.0 / D, musq[:],
                    ALU.mult, ALU.subtract)
                sd = rpool.tile([1, SH], F32, tag="sd", bufs=2)
                nc.scalar.activation(sd[:], varr[:], AF.Sqrt, bias=c_eps[:])
                r_row = rpool.tile([1, SH], F32, tag="rrow",
                                   name=f"r_{pfx}", bufs=2)
                nc.vector.reciprocal_approx_fast(r_row[:], sd[:])
                # broadcast r across partitions via K=1 matmul
                r16 = rpool.tile([1, SH], F16, tag="r16", bufs=2)
                nc.vector.tensor_copy(r16[:], r_row[:])
                bc_ps = ps_bc.tile([128, SH], F32, tag="bc", bufs=2)
                nc.tensor.matmul(bc_ps[:], ones_row[:, 0:128], r16[:],
                                 start=True, stop=True)
                rb = bc_pool.tile([128, SH], F32, tag="rb",
                                  name=f"rb_{pfx}", bufs=2)
                nc.scalar.copy(rb[:], bc_ps[:])
                return rb, nm16, r_row

            with (
                tc.tile_pool(name="wts", bufs=2) as wpool,
                tc.tile_pool(name="act", bufs=1) as apool,
                tc.tile_pool(name="scratch", bufs=2) as spool,
                tc.tile_pool(name="bcast", bufs=1) as bcpool,
                tc.tile_pool(name="ps_row", bufs=1, space="PSUM") as ps_row,
                tc.tile_pool(name="ps_bc", bufs=2, space="PSUM") as ps_bc,
                tc.tile_pool(name="ps_mm", bufs=2, space="PSUM") as ps_mm,
                tc.tile_pool(name="ps_at", bufs=1, space="PSUM") as ps_at,
            ):
                arout_m_prev = None
                for l in range(nl):
                    # ---- weight loads (double-buffered via bufs=2 pools)
                    wattn_sb = []
                    for kc in range(8):
                        w = wpool.tile([128, 3 * CW], F16, tag=f"wattn{kc}",
                                       name=f"wattn{kc}_{l}")
                        nc.sync.dma_start(
                            w[:], wattn_d[l, kc * 128:(kc + 1) * 128, :])
                        wattn_sb.append(w)
                    csqkv_sb = wpool.tile([1, 3 * CW], F16, tag="csqkv",
                                          name=f"csqkv_{l}")
                    nc.sync.dma_start(csqkv_sb[:], csqkv_d[l])
                    if has_bias_qkv:
                        bqkv_sb = wpool.tile([1, 3 * CW], F16, tag="bqkv",
                                             name=f"bqkv_{l}")
                        nc.sync.dma_start(bqkv_sb[:], bqkv_d[l])
                    wproj_sb = []
                    for t in range(2):
                        w = wpool.tile([128, D], F16, tag=f"wproj{t}",
                                       name=f"wproj{t}_{l}")
                        nc.sync.dma_start(
                            w[:], wproj_d[l, t * 128:(t + 1) * 128, :])
                        wproj_sb.append(w)
                    if has_bias_proj:
                        bproj_sb = wpool.tile([1, D], F16, tag="bproj",
                                              name=f"bproj_{l}")
                        nc.sync.dma_start(bproj_sb[:], bproj_d[l])
                    wfc_sb = []
                    for kc in range(8):
                        w = wpool.tile([128, DFS], F16, tag=f"wfc{kc}",
                                       name=f"wfc{kc}_{l}")
                        nc.sync.dma_start(
                            w[:], wfc_d[l, kc * 128:(kc + 1) * 128, :])
                        wfc_sb.append(w)
                    csfc_sb = wpool.tile([1, DFS], F16, tag="csfc",
                                         name=f"csfc_{l}")
                    nc.sync.dma_start(csfc_sb[:], csfc_d[l])
                    if has_bias_fc:
                        bfc_sb = wpool.tile([1, DFS], F16, tag="bfc",
                                            name=f"bfc_{l}")
                        nc.sync.dma_start(bfc_sb[:], bfc_d[l])
                    wfc2_sb = []
                    for kc in range(8):
                        w = wpool.tile([128, D], F16, tag=f"wfc2{kc}",
                                       name=f"wfc2{kc}_{l}")
                        nc.sync.dma_start(
                            w[:], wfc2_d[l, kc * 128:(kc + 1) * 128, :])
                        wfc2_sb.append(w)
                    if has_bias_fc2:
                        bfc2_sb = wpool.tile([1, D], F16, tag="bfc2",
                                             name=f"bfc2_{l}")
                        nc.sync.dma_start(bfc2_sb[:], bfc2_d[l])

                    # per-layer kv / q / o tiles
                    kT = [[apool.tile([128, SH], F16, tag=f"kT{t}_{hf}",
                                      name=f"kT{t}_{hf}_{l}")
                           for hf in range(2)] for t in range(2)]
                    vT = [[apool.tile([128, SH], F16, tag=f"vT{t}_{hf}",
                                      name=f"vT{t}_{hf}_{l}")
                           for hf in range(2)] for t in range(2)]
                    qT = [[apool.tile([128, SH], F16, tag=f"qT{t}_{hf}",
                                      name=f"qT{t}_{hf}_{l}")
                           for hf in range(2)] for t in range(2)]
                    oT = [[apool.tile([128, SH], F16, tag=f"oT{t}_{hf}",
                                      name=f"oT{t}_{hf}_{l}")
                           for hf in range(2)] for t in range(2)]
                    kTi, vTi = [], []
                    for t in range(2):
                        ki = apool.tile([128, 128], F16, tag=f"kTi{t}",
                                        name=f"kTi{t}_{l}")
                        nc.vector.memset(ki[:, 1:128], 0.0)
                        nc.sync.dma_start(ki[:, 0:1],
                                          kivik_d[l, t * 128:(t + 1) * 128])
                        vi = apool.tile([128, 128], F16, tag=f"vTi{t}",
                                        name=f"vTi{t}_{l}")
                        nc.vector.memset(vi[:, 1:128], 0.0)
                        nc.sync.dma_start(vi[:, 0:1],
                                          kiviv_d[l, t * 128:(t + 1) * 128])
                        kTi.append(ki)
                        vTi.append(vi)
                    # per-head v5 (transposed v + ones col), [128, 5*65]
                    v5 = [apool.tile([128, 5 * 65], F16, tag=f"v5_{h4}",
                                     name=f"v5_{h4}_{l}") for h4 in range(4)]

                    arin_a, arout_a, arin_m, arout_m = [], [], [], []
                    for hf in range(2):
                        arin_a.append(dpool.tile([D, SH], F16,
                                                 name=f"arin_a{l}_{hf}"))
                        arout_a.append(dpool.tile([D, SH], F16,
                                                  name=f"arout_a{l}_{hf}"))
                        arin_m.append(dpool.tile([D, SH], F16,
                                                 name=f"arin_m{l}_{hf}"))
                        arout_m.append(dpool.tile([D, SH], F16,
                                                  name=f"arout_m{l}_{hf}"))

                    def kv_chunk_src(c, t):
                        """(tile, col_slice) for kt chunk c of head-tile t."""
                        if c == 4:
                            return (kTi[t], vTi[t]), slice(0, 128)
                        hf_, cc = divmod(c, 2)
                        return ((kT[t][hf_], vT[t][hf_]),
                                slice(cc * 128, (cc + 1) * 128))

                    # =========== per-half attention sublayer ===========
                    for hf in range(2):
                        # residual add from previous layer's MLP AllReduce
                        if arout_m_prev is not None:
                            for kc in range(8):
                                nc.gpsimd.dma_start(
                                    hT[kc][hf][:],
                                    arout_m_prev[hf][kc * 128:(kc + 1) * 128,
                                                     :],
                                    accum_op=ALU.add)
                        rb1, nm1, _ = ln_stats(hf, f"a{l}{hf}", spool,
                                               ps_row, ps_bc, bcpool)
                        # qkv: 6 col-blocks of 128 (q0 q1 k0 k1 v0 v1)
                        for cb in range(6):
                            csl = slice(cb * 128, (cb + 1) * 128)
                            ps = ps_mm.tile([128, SH], F32, tag="mm")
                            for kc in range(8):
                                nc.tensor.matmul(
                                    ps[:], wattn_sb[kc][:, csl],
                                    hT[kc][hf][:],
                                    start=(kc == 0), stop=False)
                            last = not has_bias_qkv
                            nc.tensor.matmul(
                                ps[:], csqkv_sb[:, csl], nm1[:],
                                start=False, stop=last)
                            if has_bias_qkv:
                                nc.tensor.matmul(
                                    ps[:], bqkv_sb[:, csl], ones_row[:],
                                    start=False, stop=True)
                            kind, t = divmod(cb, 2)
                            dst = (qT, kT, vT)[kind][t][hf][:]
                            nc.vector.tensor_tensor(dst, ps[:], rb1[:],
                                                    ALU.mult)
                        # attention for this half's queries
                        chunks = [0, 1, 4] if hf == 0 else [0, 1, 2, 3, 4]
                        new_v = [0, 1, 4] if hf == 0 else [2, 3]
                        for h4 in range(4):
                            t, sub = divmod(h4, 2)
                            rsl = slice(sub * 64, (sub + 1) * 64)
                            # build v5 chunks that became available
                            vt_ps = ps_at.tile([128, 5 * 64], F16, tag="sv",
                                               bufs=2)
                            for c in new_v:
                                (kk, vv), cs = kv_chunk_src(c, t)
                                nc.tensor.transpose(
                                    vt_ps[:, c * 64:(c + 1) * 64],
                                    vv[rsl, cs], ident_sb[rsl, 0:64])
                            v5v = v5[h4].rearrange("p (c w) -> p c w", c=5)
                            vtv = vt_ps[:].rearrange("p (c w) -> p c w", c=5)
                            if hf == 0:
                                nc.scalar.copy(v5v[:, 0:2, 0:64],
                                               vtv[:, 0:2, :])
                                nc.scalar.copy(v5v[:, 4:5, 0:64],
                                               vtv[:, 4:5, :])
                                for c in range(5):
                                    nc.vector.memset(
                                        v5[h4][:, c * 65 + 64:c * 65 + 65],
                                        1.0)
                            else:
                                nc.scalar.copy(v5v[:, 2:4, 0:64],
                                               vtv[:, 2:4, :])
                            # scores -> exp -> mask -> p tiles
                            p_tiles = {}
                            for c in chunks:
                                (kk, vv), cs = kv_chunk_src(c, t)
                                sv = ps_at.tile([128, 5 * 64], F32,
                                                tag="sv", bufs=2)
                                sps = sv[:, 0:SH]
                                nc.tensor.matmul(
                                    sps, kk[rsl, cs], qT[t][hf][rsl, :],
                                    start=True, stop=True)
                                e = spool.tile([128, SH], F16,
                                               tag=f"e{c & 1}")
                                nc.scalar.activation(e[:], sps, AF.Exp)
                                p = spool.tile([128, SH], F16, tag=f"p{c}",
                                               bufs=1)
                                nc.vector.tensor_tensor(
                                    p[:], e[:],
                                    mask_sb[c][:, hf * SH:(hf + 1) * SH],
                                    ALU.mult)
                                p_tiles[c] = p
                            # AV with ones column -> [65, SH]
                            o_ps = ps_at.tile([128, SH], F32, tag="o",
                                              bufs=1)
                            for i, c in enumerate(chunks):
                                nc.tensor.matmul(
                                    o_ps[0:65, :],
                                    v5[h4][:, c * 65:(c + 1) * 65],
                                    p_tiles[c][:],
                                    start=(i == 0),
                                    stop=(i == len(chunks) - 1))
                            den = rpool.tile([1, SH], F32, tag="den")
                            nc.scalar.copy(den[:], o_ps[64:65, :])
                            rc = rpool.tile([1, SH], F32, tag="rc")
                            nc.vector.reciprocal_approx_fast(rc[:], den[:])
                            rc16 = rpool.tile([1, SH], F16, tag="rc16")
                            nc.vector.tensor_copy(rc16[:], rc[:])
                            rbo_ps = ps_bc.tile([128, SH], F32, tag="bc",
                                                bufs=2)
                            nc.tensor.matmul(rbo_ps[0:64, :],
                                             ones_row[:, 0:64], rc16[:],
                                             start=True, stop=True)
                            rbos = spool.tile([64, SH], F32, tag="rbos",
                                              bufs=2)
                            nc.scalar.copy(rbos[:], rbo_ps[0:64, :])
                            nc.vector.tensor_tensor(
                                oT[t][hf][rsl, :], o_ps[0:64, :], rbos[:],
                                ALU.mult)
                        # proj partials -> arin -> AllReduce
                        for mc in range(8):
                            msl = slice(mc * 128, (mc + 1) * 128)
                            zps = ps_mm.tile([128, SH], F32, tag="mm")
                            nc.tensor.matmul(zps[:], wproj_sb[0][:, msl],
                                             oT[0][hf][:],
                                             start=True, stop=False)
                            last = not has_bias_proj
                            nc.tensor.matmul(zps[:], wproj_sb[1][:, msl],
                                             oT[1][hf][:],
                                             start=False, stop=last)
                            if has_bias_proj:
                                nc.tensor.matmul(
                                    zps[:], bproj_sb[:, msl], ones_row[:],
                                    start=False, stop=True)
                            zsb = spool.tile([128, SH], F16, tag="ardrain",
                                             bufs=3)
                            if mc % 2 == 0:
                                nc.scalar.copy(zsb[:], zps[:])
                            else:
                                nc.vector.tensor_copy(zsb[:], zps[:])
                            nc.sync.dma_start(arin_a[hf][msl, :], zsb[:])
                        nc.gpsimd.collective_compute(
                            "AllReduce", ALU.add, replica_groups=RG,
                            ins=[arin_a[hf].opt()], outs=[arout_a[hf].opt()])

                    # =========== per-half MLP sublayer ===========
                    for hf in range(2):
                        for kc in range(8):
                            nc.gpsimd.dma_start(
                                hT[kc][hf][:],
                                arout_a[hf][kc * 128:(kc + 1) * 128, :],
                                accum_op=ALU.add)
                        rb2, nm2, _ = ln_stats(hf, f"m{l}{hf}", spool,
                                               ps_row, ps_bc, bcpool)
                        g_sb = [apool.tile([128, SH], F16,
                                           tag=f"g{mb}_{hf}",
                                           name=f"g{mb}_{hf}_{l}")
                                for mb in range(8)]
                        for mb in range(8):
                            csl = slice(mb * 128, (mb + 1) * 128)
                            ps = ps_mm.tile([128, SH], F32, tag="mm")
                            for kc in range(8):
                                nc.tensor.matmul(
                                    ps[:], wfc_sb[kc][:, csl],
                                    hT[kc][hf][:],
                                    start=(kc == 0), stop=False)
                            last = not has_bias_fc
                            nc.tensor.matmul(
                                ps[:], csfc_sb[:, csl], nm2[:],
                                start=False, stop=last)
                            if has_bias_fc:
                                nc.tensor.matmul(
                                    ps[:], bfc_sb[:, csl], ones_row[:],
                                    start=False, stop=True)
                            pre = spool.tile([128, SH], F32,
                                             tag=f"pre{mb & 1}")
                            nc.vector.tensor_tensor(pre[:], ps[:], rb2[:],
                                                    ALU.mult)
                            nc.scalar.activation(g_sb[mb][:], pre[:],
                                                 AF.Gelu_apprx_tanh)
                        for mc in range(8):
                            msl = slice(mc * 128, (mc + 1) * 128)
                            zps = ps_mm.tile([128, SH], F32, tag="mm")
                            for kc in range(8):
                                lastk = (kc == 7) and not has_bias_fc2
                                nc.tensor.matmul(
                                    zps[:], wfc2_sb[kc][:, msl],
                                    g_sb[kc][:],
                                    start=(kc == 0), stop=lastk)
                            if has_bias_fc2:
                                nc.tensor.matmul(
                                    zps[:], bfc2_sb[:, msl], ones_row[:],
                                    start=False, stop=True)
                            zsb = spool.tile([128, SH], F16, tag="ardrain",
                                             bufs=3)
                            if mc % 2 == 0:
                                nc.scalar.copy(zsb[:], zps[:])
                            else:
                                nc.vector.tensor_copy(zsb[:], zps[:])
                            nc.sync.dma_start(arin_m[hf][msl, :], zsb[:])
                        nc.gpsimd.collective_compute(
                            "AllReduce", ALU.add, replica_groups=RG,
                            ins=[arin_m[hf].opt()], outs=[arout_m[hf].opt()])
                    arout_m_prev = arout_m

            # ================= final LN + LM head =================
            with (
                tc.tile_pool(name="lm_w", bufs=2) as lwpool,
                tc.tile_pool(name="lm_x", bufs=1) as lxpool,
                tc.tile_pool(name="lm_sc", bufs=2) as lspool,
                tc.tile_pool(name="ps_lr", bufs=1, space="PSUM") as ps_lr,
                tc.tile_pool(name="ps_lbc", bufs=1, space="PSUM") as ps_lbc,
                tc.tile_pool(name="ps_lm", bufs=4, space="PSUM") as ps_lm,
            ):
                if has_bias_lm:
                    blm_sb = lwpool.tile([1, VS], F16, tag="blm",
                                         name="blm_sb", bufs=1)
                    nc.sync.dma_start(blm_sb[:], blm_d[:])
                xf = [lxpool.tile([128, S], F16, tag=f"xf{kc}",
                                  name=f"xf{kc}") for kc in range(8)]
                for hf in range(2):
                    if arout_m_prev is not None:
                        for kc in range(8):
                            nc.gpsimd.dma_start(
                                hT[kc][hf][:],
                                arout_m_prev[hf][kc * 128:(kc + 1) * 128, :],
                                accum_op=ALU.add)
                    # stats for this half
                    stat_ps = ps_lr.tile([33, SH], F32, tag="stat", bufs=1)
                    mu_ps = stat_ps[0:1, :]
                    ssq_ps = stat_ps[32:33, :]
                    for kc in range(8):
                        nc.tensor.matmul(mu_ps, ones_col[:],
                                         hT[kc][hf][:],
                                         start=(kc == 0), stop=(kc == 7))
                    for kc in range(8):
                        xsq = lspool.tile([128, SH], F16, tag=f"xsq{kc & 1}")
                        nc.gpsimd.tensor_tensor(xsq[:], hT[kc][hf][:],
                                                hT[kc][hf][:], ALU.mult)
                        nc.tensor.matmul(ssq_ps, ones_col[:], xsq[:],
                                         start=(kc == 0), stop=(kc == 7))
                    nm16 = rpool.tile([1, SH], F16, tag="nm16", bufs=2)
                    nc.scalar.mul(nm16[:], mu_ps, c_ninvD[:])
                    musq = rpool.tile([1, SH], F32, tag="musq", bufs=2)
                    nc.vector.tensor_tensor(musq[:], nm16[:], nm16[:],
                                            ALU.mult)
                    varr = rpool.tile([1, SH], F32, tag="varr", bufs=2)
                    nc.vector.scalar_tensor_tensor(
                        varr[:], ssq_ps, 1.0 / D, musq[:],
                        ALU.mult, ALU.subtract)
                    sd = rpool.tile([1, SH], F32, tag="sd", bufs=2)
                    nc.scalar.activation(sd[:], varr[:], AF.Sqrt,
                                         bias=c_eps[:])
                    r_row = rpool.tile([1, SH], F32, tag="rrow", bufs=2)
                    nc.vector.reciprocal_approx_fast(r_row[:], sd[:])
                    r16 = rpool.tile([1, SH], F16, tag="r16", bufs=2)
                    nc.vector.tensor_copy(r16[:], r_row[:])
                    # mur = (-mu) * r
                    mur = rpool.tile([1, SH], F16, tag="mur", bufs=2)
                    nc.vector.tensor_tensor(mur[:], nm16[:], r_row[:],
                                            ALU.mult)
                    rbf_ps = ps_lbc.tile([128, SH], F32, tag="rbf", bufs=1)
                    nc.tensor.matmul(rbf_ps[:], ones_row[:, 0:128], r16[:],
                                     start=True, stop=True)
                    mrb_ps = ps_lbc.tile([128, SH], F32, tag="mrb", bufs=1)
                    nc.tensor.matmul(mrb_ps[:], ones_row[:, 0:128], mur[:],
                                     start=True, stop=True)
                    rbf = lspool.tile([128, SH], F32, tag="rbf", bufs=2)
                    nc.scalar.copy(rbf[:], rbf_ps[:])
                    mrb = lspool.tile([128, SH], F32, tag="mrb", bufs=2)
                    nc.scalar.copy(mrb[:], mrb_ps[:])
                    tsl = slice(hf * SH, (hf + 1) * SH)
                    for kc in range(8):
                        if kc % 2 == 0:
                            nc.vector.tensor_tensor(
                                xf[kc][:, tsl], hT[kc][hf][:], rbf[:],
                                ALU.mult)
                            nc.vector.tensor_tensor(
                                xf[kc][:, tsl], xf[kc][:, tsl], mrb[:],
                                ALU.add)
                        else:
                            nc.gpsimd.tensor_tensor(
                                xf[kc][:, tsl], hT[kc][hf][:], rbf[:],
                                ALU.mult)
                            nc.gpsimd.tensor_tensor(
                                xf[kc][:, tsl], xf[kc][:, tsl], mrb[:],
                                ALU.add)

                for vt in range(NVT):
                    vsl = slice(vt * 512, (vt + 1) * 512)
                    wt_sb = []
                    for kc in range(8):
                        w = lwpool.tile([128, 512], F16, tag=f"wte{kc}",
                                        name=f"wte{kc}_{vt}")
                        nc.sync.dma_start(
                            w[:], wteT_d[kc * 128:(kc + 1) * 128, vsl])
                        wt_sb.append(w)
                    for tb in range(4):
                        csl = slice(tb * 128, (tb + 1) * 128)
                        lg = ps_lm.tile([128, 512], F32, tag="lg")
                        for kc in range(8):
                            lastk = (kc == 7) and not has_bias_lm
                            nc.tensor.matmul(
                                lg[:], xf[kc][:, csl], wt_sb[kc][:],
                                start=(kc == 0), stop=lastk)
                        if has_bias_lm:
                            nc.tensor.matmul(
                                lg[:], ones_row[:, 0:128], blm_sb[:, vsl],
                                start=False, stop=True)
                        lsb = lspool.tile([128, 512], F16, tag="lmdrain",
                                          bufs=4)
                        if tb % 2 == 0:
                            nc.scalar.copy(lsb[:], lg[:])
                        else:
                            nc.vector.tensor_copy(lsb[:], lg[:])
                        nc.sync.dma_start(logits_d[csl, vsl], lsb[:])

    nc.compile()
    return nc


def _prep(inputs):
    """Host-side preprocessing. Returns (in_maps, nl, bias_flags)."""
    f = lambda x: np.asarray(x, dtype=np.float32)
    ids = np.asarray(inputs["input_ids"]).astype(np.int64)
    am = f(inputs["attention_mask"])
    ihs = f(inputs["image_hidden_states"])
    wte = f(inputs["wte"])
    ft_W1, ft_b1 = f(inputs["ft_W1"]), f(inputs["ft_b1"])
    ft_W2, ft_b2 = f(inputs["ft_W2"]), f(inputs["ft_b2"])
    ln1_g, ln1_b = f(inputs["ln1_g"]), f(inputs["ln1_b"])
    Wattn, battn = f(inputs["Wattn"]), f(inputs["battn"])
    Wuk, buk = f(inputs["Wuk"]), f(inputs["buk"])
    Wuv, buv = f(inputs["Wuv"]), f(inputs["buv"])
    Wproj, bproj = f(inputs["Wproj"]), f(inputs["bproj"])
    ln2_g, ln2_b = f(inputs["ln2_g"]), f(inputs["ln2_b"])
    Wfc, bfc = f(inputs["Wfc"]), f(inputs["bfc"])
    Wfc2, bfc2 = f(inputs["Wfc2"]), f(inputs["bfc2"])
    lnf_g, lnf_b = f(inputs["lnf_g"]), f(inputs["lnf_b"])

    nl = int(os.environ.get("BASS_NLAYERS", str(L)))

    # embedding + image transform (host)
    h0 = wte[ids.reshape(-1)] + np.tile(wte[:S], (B, 1))  # [T, D]
    img = np.maximum(ihs @ ft_W1 + ft_b1, 0.0) @ ft_W2 + ft_b2  # [B, D]

    # image k/v for all layers: [nl, B, D]
    ki = np.einsum("bd,ldm->lbm", img, Wuk[:nl]) + buk[:nl][:, None, :]
    vi = np.einsum("bd,ldm->lbm", img, Wuv[:nl]) + buv[:nl][:, None, :]

    # multiplicative causal mask, tokens-first kt layout:
    # kt j<512: token j, j=512: image, j>512: pad
    # query i sees token j iff j <= i and am[j]; image always
    KT = 640
    j = np.arange(KT)
    i = np.arange(S)
    vis = (j[:, None] <= i[None, :]) & (j[:, None] < S)
    masks = np.zeros((B, KT, S), np.float32)
    for b in range(B):
        m = vis.astype(np.float32).copy()
        amb = np.concatenate([am[b], np.zeros(KT - S, np.float32)])
        m *= amb[:, None]
        m[S, :] = 1.0  # image row always visible
        masks[b] = m
    masks = masks.reshape(B, 5, 128, S)

    ident = np.tile(np.eye(HD, dtype=np.float32), (2, 2))
    qs = 1.0 / np.sqrt(np.float32(HD))

    in_maps = []
    for c in range(NC):
        g, r4 = divmod(c, TPG)
        hg = [r4 * HL + t for t in range(HL)]
        qcols = np.concatenate([np.arange(h * HD, (h + 1) * HD) for h in hg])
        kcols = D + qcols
        vcols = 2 * D + qcols

        wq = Wattn[:nl][:, :, qcols] * qs
        wk = Wattn[:nl][:, :, kcols]
        wv = Wattn[:nl][:, :, vcols]
        wqkv = np.concatenate([wq, wk, wv], axis=2)  # [nl, D, 768]
        wqkv = ln1_g[:nl][:, :, None] * wqkv
        csqkv = wqkv.sum(axis=1, keepdims=True)
        bq = battn[:nl][:, qcols] * qs
        bk = battn[:nl][:, kcols]
        bv = battn[:nl][:, vcols]
        bqkv = np.concatenate([bq, bk, bv], axis=1)[:, None, :]
        bqkv = bqkv + np.einsum("ld,ldm->lm", ln1_b[:nl],
                                np.concatenate([wq, wk, wv], axis=2)
                                )[:, None, :]

        wproj_c = np.ascontiguousarray(Wproj[:nl][:, qcols, :])
        bproj_c = (bproj[:nl] / TPG)[:, None, :]

        dsl = slice(r4 * DFS, (r4 + 1) * DFS)
        wfc_c = ln2_g[:nl][:, :, None] * Wfc[:nl][:, :, dsl]
        csfc_c = wfc_c.sum(axis=1, keepdims=True)
        bfc_c = (bfc[:nl][:, dsl][:, None, :]
                 + np.einsum("ld,ldm->lm", ln2_b[:nl],
                             Wfc[:nl][:, :, dsl])[:, None, :])
        wfc2_c = np.ascontiguousarray(Wfc2[:nl][:, dsl, :])
        bfc2_c = (bfc2[:nl] / TPG)[:, None, :]

        kivik_c = np.ascontiguousarray(ki[:, g, qcols])[:, :, None]
        kiviv_c = np.ascontiguousarray(vi[:, g, qcols])[:, :, None]

        v0 = r4 * VSH
        v1 = min(V, v0 + VSH)
        wt_rows = wte[v0:v1] * lnf_g[None, :]
        wteT_c = np.zeros((D, VS), np.float32)
        wteT_c[:, : v1 - v0] = wt_rows.T
        blm_row = lnf_b @ wte[v0:v1].T
        blm_c = np.zeros((1, VS), np.float32)
        blm_c[0, : v1 - v0] = blm_row

        h0T_c = np.ascontiguousarray(h0[g * S:(g + 1) * S].T)

        h16 = lambda x: np.ascontiguousarray(x, dtype=np.float16)
        m = {
            "h0T": h16(h0T_c),
            "wattn": h16(wqkv), "csqkv": h16(csqkv),
            "wproj": h16(wproj_c),
            "wfc": h16(wfc_c), "csfc": h16(csfc_c),
            "wfc2": h16(wfc2_c),
            "kivik": h16(kivik_c), "kiviv": h16(kiviv_c),
            "mask": h16(masks[g]), "ident": h16(ident),
            "wteT": h16(wteT_c),
        }
        m["_bqkv"] = h16(bqkv)
        m["_bproj"] = h16(bproj_c)
        m["_bfc"] = h16(bfc_c)
        m["_bfc2"] = h16(bfc2_c)
        m["_blm"] = h16(blm_c)
        in_maps.append(m)
    names = ("bqkv", "bproj", "bfc", "bfc2", "blm")
    bias_flags = tuple(
        bool(any(np.any(m["_" + n]) for m in in_maps)) for n in names)
    for m in in_maps:
        for n, flag in zip(names, bias_flags):
            arr = m.pop("_" + n)
            if flag:
                m[n] = arr
    return in_maps, nl, bias_flags


_LAST_RESULTS = {}


def kernel(**inputs):
    in_maps, nl, bias_flags = _prep(inputs)
    nc = _build(nl, *bias_flags)
    trace = bool(int(os.environ.get("BASS_KERNEL_TRACE", "0")))
    res = bass_utils.run_bass_kernel_spmd(
        nc, in_maps, core_ids=list(range(NC)), trace=trace)
    _LAST_RESULTS["res"] = res
    logits = np.empty((B, S, V), np.float32)
    for c in range(NC):
        g, r4 = divmod(c, TPG)
        v0 = r4 * VSH
        v1 = min(V, v0 + VSH)
        logits[g, :, v0:v1] = res.results[c]["logits"][:, : v1 - v0]
    return logits


# revision 20
# speedup vs baseline: 1.0778x; 1.0778x over previous
"""Trainium2 Bass kernel for nn_DecoderModel (12-layer decoder w/ image token).

Sharding: TP-4 x DP-2.
  - cores 0-3 handle batch 0, cores 4-7 batch 1 (data parallel over batch)
  - within each group of 4: Megatron TP (4 heads/core, qkv col-sharded,
    proj row-sharded, fc col-sharded 1024 dff/core, fc2 row-sharded)
  - AllReduce on replica groups [[0,1,2,3],[4,5,6,7]] -- the two groups'
    collectives run concurrently on disjoint links
  - lm head: vocab quarter per core within each batch group
  - tokens of each batch split in two 256-halves, software-pipelined so
    each AllReduce hides behind the other half's compute

Layout: residual kept feature-major (h^T: [D, tok]); LayerNorm folded into
matmuls: y = r .* (x @ Wg - mu * colsum(Wg)), Wg = gamma-folded W; the
-mu*colsum term is a K=1 rank-1 matmul into the same PSUM; r rides the
PSUM drain multiply against a PE-broadcast r row.

Attention: kv laid out tokens-first (kt 0..511 = tokens, 512 = image,
pad to 640); scores kt-major so softmax is exp + multiplicative mask;
first token-half only touches kv chunks {0,1,image}. Denominators via a
ones-column appended to transposed V (K=65 AV matmul).

Residual adds ride gpsimd accumulate-DMA (CCE add in the DMA datapath).
"""

import os
import numpy as np

from concourse import bacc, tile, mybir
from concourse import bass_utils

dt = mybir.dt
AF = mybir.ActivationFunctionType
ALU = mybir.AluOpType

# model dims
B, S, D, H, L, V = 2, 512, 1024, 16, 12, 50257
HD = D // H          # 64
DFF = 4 * D          # 4096
NC = 8
TPG = 4              # tensor-parallel group size
HL = H // TPG        # 4 local heads
CW = HL * HD         # 256 local q/k/v cols (2 partition tiles)
DFS = DFF // TPG     # 1024 local dff
SH = S // 2          # 256 tokens per pipeline half
VSH = (V + TPG - 1) // TPG   # 12565 vocab rows per core
NVT = 25                     # vocab tiles of 512
VS = NVT * 512               # 12800 padded vocab shard
EPS = 1e-5

F32 = dt.float32
F16 = dt.float16

RG = [[0, 1, 2, 3], [4, 5, 6, 7]]


def _build(nl, has_bias_qkv, has_bias_proj, has_bias_fc, has_bias_fc2,
           has_bias_lm):
    nc = bacc.Bacc("TRN2", target_bir_lowering=False, debug=False,
                   num_devices=NC)

    dram = lambda n, sh, ty=F32, kind="ExternalInput": nc.dram_tensor(
        n, sh, ty, kind=kind).ap()

    h0T_d = dram("h0T", [D, S], F16)
    wattn_d = dram("wattn", [nl, D, 3 * CW], F16)
    csqkv_d = dram("csqkv", [nl, 1, 3 * CW], F16)
    bqkv_d = dram("bqkv", [nl, 1, 3 * CW], F16) if has_bias_qkv else None
    wproj_d = dram("wproj", [nl, CW, D], F16)
    bproj_d = dram("bproj", [nl, 1, D], F16) if has_bias_proj else None
    wfc_d = dram("wfc", [nl, D, DFS], F16)
    csfc_d = dram("csfc", [nl, 1, DFS], F16)
    bfc_d = dram("bfc", [nl, 1, DFS], F16) if has_bias_fc else None
    wfc2_d = dram("wfc2", [nl, DFS, D], F16)
    bfc2_d = dram("bfc2", [nl, 1, D], F16) if has_bias_fc2 else None
    kivik_d = dram("kivik", [nl, CW, 1], F16)
    kiviv_d = dram("kiviv", [nl, CW, 1], F16)
    mask_d = dram("mask", [5, 128, S], F16)
    ident_d = dram("ident", [128, 128], F16)
    wteT_d = dram("wteT", [D, VS], F16)
    blm_d = dram("blm", [1, VS], F16) if has_bias_lm else None
    logits_d = dram("logits", [S, VS], F16, kind="ExternalOutput")

    with tile.TileContext(nc) as tc:
        with (
            nc.allow_low_precision(reason="f16 matmul pipeline"),
            tc.tile_pool(name="const", bufs=1) as cpool,
            tc.tile_pool(name="resid", bufs=1) as hpool,
            tc.tile_pool(name="rows", bufs=2) as rpool,
            tc.tile_pool(name="dram", bufs=1, space="DRAM") as dpool,
        ):
            ident_sb = cpool.tile([128, 128], F16, name="ident_sb")
            nc.sync.dma_start(ident_sb[:], ident_d[:])
            ones_col = cpool.tile([128, 1], F16, name="ones_col")
            nc.vector.memset(ones_col[:], 1.0)
            ones_row = cpool.tile([1, SH], F16, name="ones_row")
            nc.vector.memset(ones_row[:], 1.0)
            c_eps = cpool.tile([1, 1], F32, name="c_eps")
            nc.vector.memset(c_eps[:], EPS)
            c_ninvD = cpool.tile([1, 1], F32, name="c_ninvD")
            nc.vector.memset(c_ninvD[:], -1.0 / D)

            mask_sb = []
            for c in range(5):
                m = cpool.tile([128, S], F16, name=f"mask_{c}")
                nc.sync.dma_start(m[:], mask_d[c])
                mask_sb.append(m)

            # residual: [kc][hf] tiles [128, 256], feature-major
            hT = []
            for kc in range(8):
                pair = []
                for hf in range(2):
                    t_ = hpool.tile([128, SH], F16, name=f"hT{kc}_{hf}")
                    nc.sync.dma_start(
                        t_[:], h0T_d[kc * 128:(kc + 1) * 128,
                                     hf * SH:(hf + 1) * SH])
                    pair.append(t_)
                hT.append(pair)

            def ln_stats(hf, pfx, sq_pool, ps_row, ps_bc, bc_pool,
                         arout_prev=None):
                """Residual add (optional) fused with stats for one half.
                Returns (rb [128,SH] f32 SBUF bcast, nm16 [1,SH] f16,
                r_row [1,SH] f32)."""
                stat_ps = ps_row.tile([33, SH], F32, tag="stat", bufs=1)
                mu_ps = stat_ps[0:1, :]
                ssq_ps = stat_ps[32:33, :]
                for kc in range(8):
                    if arout_prev is not None:
                        z = sq_pool.tile([128, SH], F16, tag=f"z{kc & 3}",
                                         bufs=4)
                        nc.sync.dma_start(
                            z[:], arout_prev[kc * 128:(kc + 1) * 128, :])
                        eng = nc.vector if kc % 2 == 0 else nc.gpsimd
                        eng.tensor_tensor(hT[kc][hf][:], hT[kc][hf][:],
                                          z[:], ALU.add)
                    nc.tensor.matmul(mu_ps, ones_col[:], hT[kc][hf][:],
                                     start=(kc == 0), stop=(kc == 7))
                    xsq = sq_pool.tile([128, SH], F16, tag=f"xsq{kc & 1}")
                    eng2 = nc.gpsimd if kc % 2 == 0 else nc.vector
                    eng2.tensor_tensor(xsq[:], hT[kc][hf][:],
                                       hT[kc][hf][:], ALU.mult)
                    nc.tensor.matmul(ssq_ps, ones_col[:], xsq[:],
                                     start=(kc == 0), stop=(kc == 7))
                nm16 = rpool.tile([1, SH], F16, tag="nm16",
                                  name=f"nm_{pfx}", bufs=2)
                nc.scalar.mul(nm16[:], mu_ps, c_ninvD[:])
                musq = rpool.tile([1, SH], F32, tag="musq", bufs=2)
                nc.vector.tensor_tensor(musq[:], nm16[:], nm16[:], ALU.mult)
                varr = rpool.tile([1, SH], F32, tag="varr", bufs=2)
                nc.vector.scalar_tensor_tensor(
                    varr[:], ssq_ps, 1.0 / D, musq[:],
                    ALU.mult, ALU.subtract)
                sd = rpool.tile([1, SH], F32, tag="sd", bufs=2)
                nc.scalar.activation(sd[:], varr[:], AF.Sqrt, bias=c_eps[:])
                r_row = rpool.tile([1, SH], F32, tag="rrow",
                                   name=f"r_{pfx}", bufs=2)
                nc.vector.reciprocal_approx_fast(r_row[:], sd[:])
                # broadcast r across partitions via K=1 matmul
                r16 = rpool.tile([1, SH], F16, tag="r16", bufs=2)
                nc.vector.tensor_copy(r16[:], r_row[:])
                bc_ps = ps_bc.tile([128, SH], F32, tag="bc", bufs=2)
                nc.tensor.matmul(bc_ps[:], ones_row[:, 0:128], r16[:],
                                 start=True, stop=True)
                rb = bc_pool.tile([128, SH], F32, tag="rb",
                                  name=f"rb_{pfx}", bufs=2)
                nc.scalar.copy(rb[:], bc_ps[:])
                return rb, nm16, r_row

            with (
                tc.tile_pool(name="wts", bufs=2) as wpool,
                tc.tile_pool(name="act", bufs=1) as apool,
                tc.tile_pool(name="scratch", bufs=2) as spool,
                tc.tile_pool(name="bcast", bufs=1) as bcpool,
                tc.tile_pool(name="ps_row", bufs=1, space="PSUM") as ps_row,
                tc.tile_pool(name="ps_bc", bufs=2, space="PSUM") as ps_bc,
                tc.tile_pool(name="ps_mm", bufs=2, space="PSUM") as ps_mm,
                tc.tile_pool(name="ps_at", bufs=1, space="PSUM") as ps_at,
            ):
                arout_m_prev = None
                for l in range(nl):
                    # ---- weight loads (double-buffered via bufs=2 pools)
                    wattn_sb = []
                    for kc in range(8):
                        w = wpool.tile([128, 3 * CW], F16, tag=f"wattn{kc}",
                                       name=f"wattn{kc}_{l}")
                        nc.sync.dma_start(
                            w[:], wattn_d[l, kc * 128:(kc + 1) * 128, :])
                        wattn_sb.append(w)
                    csqkv_sb = wpool.tile([1, 3 * CW], F16, tag="csqkv",
                                          name=f"csqkv_{l}")
                    nc.sync.dma_start(csqkv_sb[:], csqkv_d[l])
                    if has_bias_qkv:
                        bqkv_sb = wpool.tile([1, 3 * CW], F16, tag="bqkv",
                                             name=f"bqkv_{l}")
                        nc.sync.dma_start(bqkv_sb[:], bqkv_d[l])
                    wproj_sb = []
                    for t in range(2):
                        w = wpool.tile([128, D], F16, tag=f"wproj{t}",
                                       name=f"wproj{t}_{l}")
                        nc.sync.dma_start(
                            w[:], wproj_d[l, t * 128:(t + 1) * 128, :])
                        wproj_sb.append(w)
                    if has_bias_proj:
                        bproj_sb = wpool.tile([1, D], F16, tag="bproj",
                                              name=f"bproj_{l}")
                        nc.sync.dma_start(bproj_sb[:], bproj_d[l])
                    wfc_sb = []
                    for kc in range(8):
                        w = wpool.tile([128, DFS], F16, tag=f"wfc{kc}",
                                       name=f"wfc{kc}_{l}")
                        nc.sync.dma_start(
                            w[:], wfc_d[l, kc * 128:(kc + 1) * 128, :])
                        wfc_sb.append(w)
                    csfc_sb = wpool.tile([1, DFS], F16, tag="csfc",
                                         name=f"csfc_{l}")
                    nc.sync.dma_start(csfc_sb[:], csfc_d[l])
                    if has_bias_fc:
                        bfc_sb = wpool.tile([1, DFS], F16, tag="bfc",
                                            name=f"bfc_{l}")
                        nc.sync.dma_start(bfc_sb[:], bfc_d[l])
                    wfc2_sb = []
                    for kc in range(8):
                        w = wpool.tile([128, D], F16, tag=f"wfc2{kc}",
                                       name=f"wfc2{kc}_{l}")
                        nc.sync.dma_start(
                            w[:], wfc2_d[l, kc * 128:(kc + 1) * 128, :])
                        wfc2_sb.append(w)
                    if has_bias_fc2:
                        bfc2_sb = wpool.tile([1, D], F16, tag="bfc2",
                                             name=f"bfc2_{l}")
                        nc.sync.dma_start(bfc2_sb[:], bfc2_d[l])

                    # per-layer kv / q / o tiles
                    kT = [[apool.tile([128, SH], F16, tag=f"kT{t}_{hf}",
                                      name=f"kT{t}_{hf}_{l}")
                           for hf in range(2)] for t in range(2)]
                    vT = [[apool.tile([128, SH], F16, tag=f"vT{t}_{hf}",
                                      name=f"vT{t}_{hf}_{l}")
                           for hf in range(2)] for t in range(2)]
                    qT = [[apool.tile([128, SH], F16, tag=f"qT{t}_{hf}",
                                      name=f"qT{t}_{hf}_{l}")
                           for hf in range(2)] for t in range(2)]
                    oT = [[apool.tile([128, SH], F16, tag=f"oT{t}_{hf}",
                                      name=f"oT{t}_{hf}_{l}")
                           for hf in range(2)] for t in range(2)]
                    kTi, vTi = [], []
                    for t in range(2):
                        ki = apool.tile([128, 128], F16, tag=f"kTi{t}",
                                        name=f"kTi{t}_{l}")
                        nc.vector.memset(ki[:, 1:128], 0.0)
                        nc.sync.dma_start(ki[:, 0:1],
                                          kivik_d[l, t * 128:(t + 1) * 128])
                        vi = apool.tile([128, 128], F16, tag=f"vTi{t}",
                                        name=f"vTi{t}_{l}")
                        nc.vector.memset(vi[:, 1:128], 0.0)
                        nc.sync.dma_start(vi[:, 0:1],
                                          kiviv_d[l, t * 128:(t + 1) * 128])
                        kTi.append(ki)
                        vTi.append(vi)
                    # per-head v5 (transposed v + ones col), [128, 5*65]
                    v5 = [apool.tile([128, 5 * 65], F16, tag=f"v5_{h4}",
                                     name=f"v5_{h4}_{l}") for h4 in range(4)]

                    arin_a, arout_a, arin_m, arout_m = [], [], [], []
                    for hf in range(2):
                        arin_a.append(dpool.tile([D, SH], F16,
                                                 name=f"arin_a{l}_{hf}"))
                        arout_a.append(dpool.tile([D, SH], F16,
                                                  name=f"arout_a{l}_{hf}"))
                        arin_m.append(dpool.tile([D, SH], F16,
                                                 name=f"arin_m{l}_{hf}"))
                        arout_m.append(dpool.tile([D, SH], F16,
                                                  name=f"arout_m{l}_{hf}"))

                    def kv_chunk_src(c, t):
                        """(tile, col_slice) for kt chunk c of head-tile t."""
                        if c == 4:
                            return (kTi[t], vTi[t]), slice(0, 128)
                        hf_, cc = divmod(c, 2)
                        return ((kT[t][hf_], vT[t][hf_]),
                                slice(cc * 128, (cc + 1) * 128))

                    # =========== per-half attention sublayer ===========
                    for hf in range(2):
                        rb1, nm1, _ = ln_stats(
                            hf, f"a{l}{hf}", spool, ps_row, ps_bc, bcpool,
                            arout_prev=(None if arout_m_prev is None
                                        else arout_m_prev[hf]))
                        # qkv: 6 col-blocks of 128 (q0 q1 k0 k1 v0 v1)
                        for cb in range(6):
                            csl = slice(cb * 128, (cb + 1) * 128)
                            ps = ps_mm.tile([128, SH], F32, tag="mm")
                            for kc in range(8):
                                nc.tensor.matmul(
                                    ps[:], wattn_sb[kc][:, csl],
                                    hT[kc][hf][:],
                                    start=(kc == 0), stop=False)
                            last = not has_bias_qkv
                            nc.tensor.matmul(
                                ps[:], csqkv_sb[:, csl], nm1[:],
                                start=False, stop=last)
                            if has_bias_qkv:
                                nc.tensor.matmul(
                                    ps[:], bqkv_sb[:, csl], ones_row[:],
                                    start=False, stop=True)
                            kind, t = divmod(cb, 2)
                            dst = (qT, kT, vT)[kind][t][hf][:]
                            nc.vector.tensor_tensor(dst, ps[:], rb1[:],
                                                    ALU.mult)
                        # attention for this half's queries
                        chunks = [0, 1, 4] if hf == 0 else [0, 1, 2, 3, 4]
                        new_v = [0, 1, 4] if hf == 0 else [2, 3]
                        for h4 in range(4):
                            t, sub = divmod(h4, 2)
                            rsl = slice(sub * 64, (sub + 1) * 64)
                            # build v5 chunks that became available
                            vt_ps = ps_at.tile([128, 5 * 64], F16, tag="sv",
                                               bufs=2)
                            for c in new_v:
                                (kk, vv), cs = kv_chunk_src(c, t)
                                nc.tensor.transpose(
                                    vt_ps[:, c * 64:(c + 1) * 64],
                                    vv[rsl, cs], ident_sb[rsl, 0:64])
                            v5v = v5[h4].rearrange("p (c w) -> p c w", c=5)
                            vtv = vt_ps[:].rearrange("p (c w) -> p c w", c=5)
                            ceng = nc.scalar if h4 % 2 == 0 else nc.vector
                            ccopy = (nc.scalar.copy if h4 % 2 == 0
                                     else nc.vector.tensor_copy)
                            if hf == 0:
                                ccopy(v5v[:, 0:2, 0:64], vtv[:, 0:2, :])
                                ccopy(v5v[:, 4:5, 0:64], vtv[:, 4:5, :])
                                for c in range(5):
                                    nc.vector.memset(
                                        v5[h4][:, c * 65 + 64:c * 65 + 65],
                                        1.0)
                            else:
                                ccopy(v5v[:, 2:4, 0:64], vtv[:, 2:4, :])
                            # scores -> exp -> mask -> p tiles
                            p_tiles = {}
                            for c in chunks:
                                (kk, vv), cs = kv_chunk_src(c, t)
                                sv = ps_at.tile([128, 5 * 64], F32,
                                                tag="sv", bufs=2)
                                sps = sv[:, 0:SH]
                                nc.tensor.matmul(
                                    sps, kk[rsl, cs], qT[t][hf][rsl, :],
                                    start=True, stop=True)
                                e = spool.tile([128, SH], F16,
                                               tag=f"e{c & 1}")
                                nc.scalar.activation(e[:], sps, AF.Exp)
                                p = spool.tile([128, SH], F16, tag=f"p{c}",
                                               bufs=1)
                                nc.vector.tensor_tensor(
                                    p[:], e[:],
                                    mask_sb[c][:, hf * SH:(hf + 1) * SH],
                                    ALU.mult)
                                p_tiles[c] = p
                            # AV with ones column -> [65, SH]
                            o_ps = ps_mm.tile([128, SH], F32, tag="mm")
                            for i, c in enumerate(chunks):
                                nc.tensor.matmul(
                                    o_ps[0:65, :],
                                    v5[h4][:, c * 65:(c + 1) * 65],
                                    p_tiles[c][:],
                                    start=(i == 0),
                                    stop=(i == len(chunks) - 1))
                            den = rpool.tile([1, SH], F32, tag="den")
                            nc.vector.tensor_copy(den[:], o_ps[64:65, :])
                            rc = rpool.tile([1, SH], F32, tag="rc")
                            nc.vector.reciprocal_approx_fast(rc[:], den[:])
                            rc16 = rpool.tile([1, SH], F16, tag="rc16")
                            nc.vector.tensor_copy(rc16[:], rc[:])
                            rbo_ps = ps_bc.tile([128, SH], F32, tag="bc",
                                                bufs=2)
                            nc.tensor.matmul(rbo_ps[0:64, :],
                                             ones_row[:, 0:64], rc16[:],
                                             start=True, stop=True)
                            rbos = spool.tile([64, SH], F32, tag="rbos",
                                              bufs=2)
                            nc.scalar.copy(rbos[:], rbo_ps[0:64, :])
                            nc.vector.tensor_tensor(
                                oT[t][hf][rsl, :], o_ps[0:64, :], rbos[:],
                                ALU.mult)
                        # proj partials -> arin -> AllReduce
                        for mc in range(8):
                            msl = slice(mc * 128, (mc + 1) * 128)
                            zps = ps_mm.tile([128, SH], F32, tag="mm")
                            nc.tensor.matmul(zps[:], wproj_sb[0][:, msl],
                                             oT[0][hf][:],
                                             start=True, stop=False)
                            last = not has_bias_proj
                            nc.tensor.matmul(zps[:], wproj_sb[1][:, msl],
                                             oT[1][hf][:],
                                             start=False, stop=last)
                            if has_bias_proj:
                                nc.tensor.matmul(
                                    zps[:], bproj_sb[:, msl], ones_row[:],
                                    start=False, stop=True)
                            zsb = spool.tile([128, SH], F16, tag="ardrain",
                                             bufs=3)
                            if mc % 2 == 0:
                                nc.scalar.copy(zsb[:], zps[:])
                            else:
                                nc.vector.tensor_copy(zsb[:], zps[:])
                            nc.sync.dma_start(arin_a[hf][msl, :], zsb[:])
                        nc.gpsimd.collective_compute(
                            "AllReduce", ALU.add, replica_groups=RG,
                            ins=[arin_a[hf].opt()], outs=[arout_a[hf].opt()])

                    # =========== per-half MLP sublayer ===========
                    for hf in range(2):
                        rb2, nm2, _ = ln_stats(hf, f"m{l}{hf}", spool,
                                               ps_row, ps_bc, bcpool,
                                               arout_prev=arout_a[hf])
                        g_sb = [apool.tile([128, SH], F16,
                                           tag=f"g{mb}_{hf}",
                                           name=f"g{mb}_{hf}_{l}")
                                for mb in range(8)]
                        for mb in range(8):
                            csl = slice(mb * 128, (mb + 1) * 128)
                            ps = ps_mm.tile([128, SH], F32, tag="mm")
                            for kc in range(8):
                                nc.tensor.matmul(
                                    ps[:], wfc_sb[kc][:, csl],
                                    hT[kc][hf][:],
                                    start=(kc == 0), stop=False)
                            last = not has_bias_fc
                            nc.tensor.matmul(
                                ps[:], csfc_sb[:, csl], nm2[:],
                                start=False, stop=last)
                            if has_bias_fc:
                                nc.tensor.matmul(
                                    ps[:], bfc_sb[:, csl], ones_row[:],
                                    start=False, stop=True)
                            pre = spool.tile([128, SH], F32,
                                             tag=f"pre{mb & 1}")
                            nc.vector.tensor_tensor(pre[:], ps[:], rb2[:],
                                                    ALU.mult)
                            nc.scalar.activation(g_sb[mb][:], pre[:],
                                                 AF.Gelu_apprx_tanh)
                        for mc in range(8):
                            msl = slice(mc * 128, (mc + 1) * 128)
                            zps = ps_mm.tile([128, SH], F32, tag="mm")
                            for kc in range(8):
                                lastk = (kc == 7) and not has_bias_fc2
                                nc.tensor.matmul(
                                    zps[:], wfc2_sb[kc][:, msl],
                                    g_sb[kc][:],
                                    start=(kc == 0), stop=lastk)
                            if has_bias_fc2:
                                nc.tensor.matmul(
                                    zps[:], bfc2_sb[:, msl], ones_row[:],
                                    start=False, stop=True)
                            zsb = spool.tile([128, SH], F16, tag="ardrain",
                                             bufs=3)
                            if mc % 2 == 0:
                                nc.scalar.copy(zsb[:], zps[:])
                            else:
                                nc.vector.tensor_copy(zsb[:], zps[:])
                            nc.sync.dma_start(arin_m[hf][msl, :], zsb[:])
                        nc.gpsimd.collective_compute(
                            "AllReduce", ALU.add, replica_groups=RG,
                            ins=[arin_m[hf].opt()], outs=[arout_m[hf].opt()])
                    arout_m_prev = arout_m

            # ================= final LN + LM head =================
            with (
                tc.tile_pool(name="lm_w", bufs=2) as lwpool,
                tc.tile_pool(name="lm_x", bufs=1) as lxpool,
                tc.tile_pool(name="lm_sc", bufs=2) as lspool,
                tc.tile_pool(name="ps_lr", bufs=1, space="PSUM") as ps_lr,
                tc.tile_pool(name="ps_lbc", bufs=1, space="PSUM") as ps_lbc,
                tc.tile_pool(name="ps_lm", bufs=4, space="PSUM") as ps_lm,
            ):
                if has_bias_lm:
                    blm_sb = lwpool.tile([1, VS], F16, tag="blm",
                                         name="blm_sb", bufs=1)
                    nc.sync.dma_start(blm_sb[:], blm_d[:])
                xf = [lxpool.tile([128, S], F16, tag=f"xf{kc}",
                                  name=f"xf{kc}") for kc in range(8)]
                for hf in range(2):
                    if arout_m_prev is not None:
                        for kc in range(8):
                            nc.gpsimd.dma_start(
                                hT[kc][hf][:],
                                arout_m_prev[hf][kc * 128:(kc + 1) * 128, :],
                                accum_op=ALU.add)
                    # stats for this half
                    stat_ps = ps_lr.tile([33, SH], F32, tag="stat", bufs=1)
                    mu_ps = stat_ps[0:1, :]
                    ssq_ps = stat_ps[32:33, :]
                    for kc in range(8):
                        nc.tensor.matmul(mu_ps, ones_col[:],
                                         hT[kc][hf][:],
                                         start=(kc == 0), stop=(kc == 7))
                    for kc in range(8):
                        xsq = lspool.tile([128, SH], F16, tag=f"xsq{kc & 1}")
                        nc.gpsimd.tensor_tensor(xsq[:], hT[kc][hf][:],
                                                hT[kc][hf][:], ALU.mult)
                        nc.tensor.matmul(ssq_ps, ones_col[:], xsq[:],
                                         start=(kc == 0), stop=(kc == 7))
                    nm16 = rpool.tile([1, SH], F16, tag="nm16", bufs=2)
                    nc.scalar.mul(nm16[:], mu_ps, c_ninvD[:])
                    musq = rpool.tile([1, SH], F32, tag="musq", bufs=2)
                    nc.vector.tensor_tensor(musq[:], nm16[:], nm16[:],
                                            ALU.mult)
                    varr = rpool.tile([1, SH], F32, tag="varr", bufs=2)
                    nc.vector.scalar_tensor_tensor(
                        varr[:], ssq_ps, 1.0 / D, musq[:],
                        ALU.mult, ALU.subtract)
                    sd = rpool.tile([1, SH], F32, tag="sd", bufs=2)
                    nc.scalar.activation(sd[:], varr[:], AF.Sqrt,
                                         bias=c_eps[:])
                    r_row = rpool.tile([1, SH], F32, tag="rrow", bufs=2)
                    nc.vector.reciprocal_approx_fast(r_row[:], sd[:])
                    r16 = rpool.tile([1, SH], F16, tag="r16", bufs=2)
                    nc.vector.tensor_copy(r16[:], r_row[:])
                    # mur = (-mu) * r
                    mur = rpool.tile([1, SH], F16, tag="mur", bufs=2)
                    nc.vector.tensor_tensor(mur[:], nm16[:], r_row[:],
                                            ALU.mult)
                    rbf_ps = ps_lbc.tile([128, SH], F32, tag="rbf", bufs=1)
                    nc.tensor.matmul(rbf_ps[:], ones_row[:, 0:128], r16[:],
                                     start=True, stop=True)
                    mrb_ps = ps_lbc.tile([128, SH], F32, tag="mrb", bufs=1)
                    nc.tensor.matmul(mrb_ps[:], ones_row[:, 0:128], mur[:],
                                     start=True, stop=True)
                    rbf = lspool.tile([128, SH], F32, tag="rbf", bufs=2)
                    nc.scalar.copy(rbf[:], rbf_ps[:])
                    mrb = lspool.tile([128, SH], F32, tag="mrb", bufs=2)
                    nc.scalar.copy(mrb[:], mrb_ps[:])
                    tsl = slice(hf * SH, (hf + 1) * SH)
                    for kc in range(8):
                        if kc % 2 == 0:
                            nc.vector.tensor_tensor(
                                xf[kc][:, tsl], hT[kc][hf][:], rbf[:],
                                ALU.mult)
                            nc.vector.tensor_tensor(
                                xf[kc][:, tsl], xf[kc][:, tsl], mrb[:],
                                ALU.add)
                        else:
                            nc.gpsimd.tensor_tensor(
                                xf[kc][:, tsl], hT[kc][hf][:], rbf[:],
                                ALU.mult)
                            nc.gpsimd.tensor_tensor(
                                xf[kc][:, tsl], xf[kc][:, tsl], mrb[:],
                                ALU.add)

                for vt in range(NVT):
                    vsl = slice(vt * 512, (vt + 1) * 512)
                    wt_sb = []
                    for kc in range(8):
                        w = lwpool.tile([128, 512], F16, tag=f"wte{kc}",
                                        name=f"wte{kc}_{vt}")
                        nc.sync.dma_start(
                            w[:], wteT_d[kc * 128:(kc + 1) * 128, vsl])
                        wt_sb.append(w)
                    for tb in range(4):
                        csl = slice(tb * 128, (tb + 1) * 128)
                        lg = ps_lm.tile([128, 512], F32, tag="lg")
                        for kc in range(8):
                            lastk = (kc == 7) and not has_bias_lm
                            nc.tensor.matmul(
                                lg[:], xf[kc][:, csl], wt_sb[kc][:],
                                start=(kc == 0), stop=lastk)
                        if has_bias_lm:
                            nc.tensor.matmul(
                                lg[:], ones_row[:, 0:128], blm_sb[:, vsl],
                                start=False, stop=True)
                        lsb = lspool.tile([128, 512], F16, tag="lmdrain",
                                          bufs=4)
                        if tb % 2 == 0:
                            nc.scalar.copy(lsb[:], lg[:])
                        else:
                            nc.vector.tensor_copy(lsb[:], lg[:])
                        nc.sync.dma_start(logits_d[csl, vsl], lsb[:])

    nc.compile()
    return nc


def _prep(inputs):
    """Host-side preprocessing. Returns (in_maps, nl, bias_flags)."""
    f = lambda x: np.asarray(x, dtype=np.float32)
    ids = np.asarray(inputs["input_ids"]).astype(np.int64)
    am = f(inputs["attention_mask"])
    ihs = f(inputs["image_hidden_states"])
    wte = f(inputs["wte"])
    ft_W1, ft_b1 = f(inputs["ft_W1"]), f(inputs["ft_b1"])
    ft_W2, ft_b2 = f(inputs["ft_W2"]), f(inputs["ft_b2"])
    ln1_g, ln1_b = f(inputs["ln1_g"]), f(inputs["ln1_b"])
    Wattn, battn = f(inputs["Wattn"]), f(inputs["battn"])
    Wuk, buk = f(inputs["Wuk"]), f(inputs["buk"])
    Wuv, buv = f(inputs["Wuv"]), f(inputs["buv"])
    Wproj, bproj = f(inputs["Wproj"]), f(inputs["bproj"])
    ln2_g, ln2_b = f(inputs["ln2_g"]), f(inputs["ln2_b"])
    Wfc, bfc = f(inputs["Wfc"]), f(inputs["bfc"])
    Wfc2, bfc2 = f(inputs["Wfc2"]), f(inputs["bfc2"])
    lnf_g, lnf_b = f(inputs["lnf_g"]), f(inputs["lnf_b"])

    nl = int(os.environ.get("BASS_NLAYERS", str(L)))

    # embedding + image transform (host)
    h0 = wte[ids.reshape(-1)] + np.tile(wte[:S], (B, 1))  # [T, D]
    img = np.maximum(ihs @ ft_W1 + ft_b1, 0.0) @ ft_W2 + ft_b2  # [B, D]

    # image k/v for all layers: [nl, B, D]
    ki = np.einsum("bd,ldm->lbm", img, Wuk[:nl]) + buk[:nl][:, None, :]
    vi = np.einsum("bd,ldm->lbm", img, Wuv[:nl]) + buv[:nl][:, None, :]

    # multiplicative causal mask, tokens-first kt layout:
    # kt j<512: token j, j=512: image, j>512: pad
    # query i sees token j iff j <= i and am[j]; image always
    KT = 640
    j = np.arange(KT)
    i = np.arange(S)
    vis = (j[:, None] <= i[None, :]) & (j[:, None] < S)
    masks = np.zeros((B, KT, S), np.float32)
    for b in range(B):
        m = vis.astype(np.float32).copy()
        amb = np.concatenate([am[b], np.zeros(KT - S, np.float32)])
        m *= amb[:, None]
        m[S, :] = 1.0  # image row always visible
        masks[b] = m
    masks = masks.reshape(B, 5, 128, S)

    ident = np.tile(np.eye(HD, dtype=np.float32), (2, 2))
    qs = 1.0 / np.sqrt(np.float32(HD))

    in_maps = []
    for c in range(NC):
        g, r4 = divmod(c, TPG)
        hg = [r4 * HL + t for t in range(HL)]
        qcols = np.concatenate([np.arange(h * HD, (h + 1) * HD) for h in hg])
        kcols = D + qcols
        vcols = 2 * D + qcols

        wq = Wattn[:nl][:, :, qcols] * qs
        wk = Wattn[:nl][:, :, kcols]
        wv = Wattn[:nl][:, :, vcols]
        wqkv = np.concatenate([wq, wk, wv], axis=2)  # [nl, D, 768]
        wqkv = ln1_g[:nl][:, :, None] * wqkv
        csqkv = wqkv.sum(axis=1, keepdims=True)
        bq = battn[:nl][:, qcols] * qs
        bk = battn[:nl][:, kcols]
        bv = battn[:nl][:, vcols]
        bqkv = np.concatenate([bq, bk, bv], axis=1)[:, None, :]
        bqkv = bqkv + np.einsum("ld,ldm->lm", ln1_b[:nl],
                                np.concatenate([wq, wk, wv], axis=2)
                                )[:, None, :]

        wproj_c = np.ascontiguousarray(Wproj[:nl][:, qcols, :])
        bproj_c = (bproj[:nl] / TPG)[:, None, :]

        dsl = slice(r4 * DFS, (r4 + 1) * DFS)
        wfc_c = ln2_g[:nl][:, :, None] * Wfc[:nl][:, :, dsl]
        csfc_c = wfc_c.sum(axis=1, keepdims=True)
        bfc_c = (bfc[:nl][:, dsl][:, None, :]
                 + np.einsum("ld,ldm->lm", ln2_b[:nl],
                             Wfc[:nl][:, :, dsl])[:, None, :])
        wfc2_c = np.ascontiguousarray(Wfc2[:nl][:, dsl, :])
        bfc2_c = (bfc2[:nl] / TPG)[:, None, :]

        kivik_c = np.ascontiguousarray(ki[:, g, qcols])[:, :, None]
        kiviv_c = np.ascontiguousarray(vi[:, g, qcols])[:, :, None]

        v0 = r4 * VSH
        v1 = min(V, v0 + VSH)
        wt_rows = wte[v0:v1] * lnf_g[None, :]
        wteT_c = np.zeros((D, VS), np.float32)
        wteT_c[:, : v1 - v0] = wt_rows.T
        blm_row = lnf_b @ wte[v0:v1].T
        blm_c = np.zeros((1, VS), np.float32)
        blm_c[0, : v1 - v0] = blm_row

        h0T_c = np.ascontiguousarray(h0[g * S:(g + 1) * S].T)

        h16 = lambda x: np.ascontiguousarray(x, dtype=np.float16)
        m = {
            "h0T": h16(h0T_c),
            "wattn": h16(wqkv), "csqkv": h16(csqkv),
            "wproj": h16(wproj_c),
            "wfc": h16(wfc_c), "csfc": h16(csfc_c),
            "wfc2": h16(wfc2_c),
            "kivik": h16(kivik_c), "kiviv": h16(kiviv_c),
            "mask": h16(masks[g]), "ident": h16(ident),
            "wteT": h16(wteT_c),
        }
        m["_bqkv"] = h16(bqkv)
        m["_bproj"] = h16(bproj_c)
        m["_bfc"] = h16(bfc_c)
        m["_bfc2"] = h16(bfc2_c)
        m["_blm"] = h16(blm_c)
        in_maps.append(m)
    names = ("bqkv", "bproj", "bfc", "bfc2", "blm")
    bias_flags = tuple(
        bool(any(np.any(m["_" + n]) for m in in_maps)) for n in names)
    for m in in_maps:
        for n, flag in zip(names, bias_flags):
            arr = m.pop("_" + n)
            if flag:
                m[n] = arr
    return in_maps, nl, bias_flags


_LAST_RESULTS = {}


def kernel(**inputs):
    in_maps, nl, bias_flags = _prep(inputs)
    nc = _build(nl, *bias_flags)
    trace = bool(int(os.environ.get("BASS_KERNEL_TRACE", "0")))
    res = bass_utils.run_bass_kernel_spmd(
        nc, in_maps, core_ids=list(range(NC)), trace=trace)
    _LAST_RESULTS["res"] = res
    logits = np.empty((B, S, V), np.float32)
    for c in range(NC):
        g, r4 = divmod(c, TPG)
        v0 = r4 * VSH
        v1 = min(V, v0 + VSH)
        logits[g, :, v0:v1] = res.results[c]["logits"][:, : v1 - v0]
    return logits
